# revision 1
# baseline (speedup 1.0000x reference)
"""BarlowTwins-style loss kernel for Trainium2 (raw Bass), 8-core SPMD.

Math: the reference materializes a (B, D, D) per-sample cross-correlation
tensor, but the loss algebraically reduces to O(B*D) work.  With
z1n/z2n the per-dim (batch-)normalized inputs and per-sample b:
    w    = z1n[b,:] * z2n[b,:]
    R    = sum(w);  P = sum(w^2);  Sa = sum(z1n^2);  Sv = sum(z2n^2)
    a    = z1n[b,b];  v = z2n[b,b];  d = a*v;  g2 = (d-1)^2
    u    = (a*z2n[b,:] - 1)^2;  Q = sum(u^2);  (g2 == u[b])
    T    = P - 2R + D                  # sum((w-1)^2)
    on   = T - g2 + (g2-1)^2
    off  = (Sa - a^2)*Sv - P + d^2 + Q - g2^2
    loss = on + 0.005*off

Sharding: data-parallel over batch.  Every core loads the full z1/z2 to
compute per-dim column sums/sumsq locally (cheaper than an all-reduce at
this size), then computes the loss for its own 16 samples using a
rearranged layout [128 partitions = (sample, col-chunk), 128 free] so all
row-reductions run at full partition occupancy.

Written in raw Bass (explicit semaphores): the walrus build in this
container only supports a single sync-wait per instruction, which the
Tile framework's exit sequence violates; standalone wait_ge instructions
compose freely.
"""

import sys
from contextlib import ExitStack

import numpy as np

for _p in ("/opt/trn_rl_repo",):
    if _p not in sys.path:
        sys.path.append(_p)

import concourse.bass as bass
import concourse.mybir as mybir
from concourse.bass_utils import run_bass_kernel_spmd

B, D = 128, 1024
NCORES = 8
SPC = B // NCORES  # 16 samples per core
KCH = D // 128     # 8 column chunks of 128
LAM = 0.005

FP = mybir.dt.float32
BF = mybir.dt.bfloat16
AF = mybir.ActivationFunctionType
AL = mybir.AluOpType

# consts column layout
C_AMASK = 0
C_EXPAND = 128
C_Z1R = 256
C_Z2R = 384
C_GSEL = 512
C_SELZ1 = 528
C_SELZ2 = 656
C_TOTAL = 784


def build_program():
    nc = bass.Bass("TRN2", debug=False, num_devices=NCORES,
                   detect_race_conditions=False)

    z1_d = nc.dram_tensor("z1", [B, D], FP, kind="ExternalInput")
    z2_d = nc.dram_tensor("z2", [B, D], FP, kind="ExternalInput")
    consts_d = nc.dram_tensor("consts", [128, C_TOTAL], FP, kind="ExternalInput")
    loss_d = nc.dram_tensor("loss", [SPC, 1], FP, kind="ExternalOutput")

    ctx = ExitStack()
    with ctx:
        sem = {n: ctx.enter_context(nc.semaphore(n)) for n in
               ["sca", "dz0", "dz1", "dz2", "dz3", "sscat", "qscat", "dout",
                "spe", "sv", "sa", "sg"]}

        def sb(name, shape, dtype=FP):
            return ctx.enter_context(nc.sbuf_tensor(name, shape, dtype))

        ca = sb("ca", [128, C_TOTAL])
        z = sb("z", [128, 2 * D])
        zb = sb("zb", [128, 2 * D], BF)
        sqb = sb("sqb", [128, 2 * D], BF)
        sumrow = sb("sumrow", [1, 2 * D])
        qrow = sb("qrow", [1, 2 * D])
        sum16 = sb("sum16", [16, 128])
        q16s = sb("q16s", [16, 128])
        q127 = sb("q127", [16, 128])
        t1 = sb("t1", [16, 128])
        var16 = sb("var16", [16, 128])
        std16 = sb("std16", [16, 128])
        A_z = sb("A_z", [16, 128])
        A_zb = sb("A_zb", [16, 128], BF)
        C_zb = sb("C_zb", [16, 128], BF)
        selz1b = sb("selz1b", [16, 128], BF)
        selz2b = sb("selz2b", [16, 128], BF)
        tn1 = sb("tn1", [128, 128])
        z1n = sb("z1n", [128, 128])
        tn2 = sb("tn2", [128, 128])
        z2n = sb("z2n", [128, 128])
        w = sb("w", [128, 128])
        u = sb("u", [128, 128])
        junkP = sb("junkP", [128, 128])
        junkQ = sb("junkQ", [128, 128])
        junkA = sb("junkA", [128, 128])
        junkV = sb("junkV", [128, 128])
        junkG = sb("junkG", [128, 128])
        junkA2 = sb("junkA2", [128, 128])
        junkV2 = sb("junkV2", [128, 128])
        negone = sb("negone", [128, 1])
        ones_b = sb("ones_b", [128, 1], BF)
        acol = sb("acol", [128, 1])
        vcol = sb("vcol", [128, 1])
        a_sb = sb("a_sb", [128, 1])
        colsD = sb("colsD", [128, 8])
        q16sb = sb("q16sb", [16, 8])
        fin = sb("fin", [16, 14])
        loss16 = sb("loss16", [16, 1])

        # PSUM: 4 stat banks (reused by blocks 2,3) + broadcast bank +
        # a-expand bank + group-reduce bank = 7 of 8 banks.
        ps = [ctx.enter_context(nc.psum_tensor(f"ps{i}", [1, 512], FP))
              for i in range(4)]
        ps_s_t = [ps[0], ps[2], ps[0], ps[2]]
        ps_q_t = [ps[1], ps[3], ps[1], ps[3]]
        psBC1 = ctx.enter_context(nc.psum_tensor("psBC1", [128, 256], FP))
        psBC2 = ctx.enter_context(nc.psum_tensor("psBC2", [128, 256], FP))
        q16 = ctx.enter_context(nc.psum_tensor("q16", [16, 8], FP))

        psA1 = psBC1[:, 0:128]
        psC1 = psBC1[:, 128:256]
        psA2 = psBC2[:, 0:128]
        psC2 = psBC2[:, 128:256]

        amask = ca[:, C_AMASK:C_AMASK + 128]
        expand = ca[:, C_EXPAND:C_EXPAND + 128]
        z1r = ca[:, C_Z1R:C_Z1R + 128]
        z2r = ca[:, C_Z2R:C_Z2R + 128]
        gsel = ca[:, C_GSEL:C_GSEL + 16]
        selz1 = ca[0:16, C_SELZ1:C_SELZ1 + 128]
        selz2 = ca[0:16, C_SELZ2:C_SELZ2 + 128]

        blksl = [slice(i * 512, (i + 1) * 512) for i in range(4)]

        with nc.Block() as block:

            @block.sync
            def _(sync):
                sync.dma_start(z[:, blksl[0]], z1_d[:, 0:512]).then_inc(sem["dz0"], 16)
                sync.dma_start(z[:, blksl[1]], z1_d[:, 512:1024]).then_inc(sem["dz1"], 16)
                sync.dma_start(z[:, blksl[2]], z2_d[:, 0:512]).then_inc(sem["dz2"], 16)
                sync.dma_start(z[:, blksl[3]], z2_d[:, 512:1024]).then_inc(sem["dz3"], 16)
                sync.dma_start(ca[:], consts_d[:]).then_inc(sem["sca"], 16)
                sync.wait_ge(sem["sv"], 10)
                sync.dma_start(sum16[:], sumrow[:]).then_inc(sem["sscat"], 16)
                sync.wait_ge(sem["sa"], 6)
                sync.dma_start(q16s[:], qrow[:]).then_inc(sem["qscat"], 16)
                sync.wait_ge(sem["sv"], 51)
                sync.dma_start(loss_d[:], loss16[:]).then_inc(sem["dout"], 16)

            @block.gpsimd
            def _(gp):
                gp.wait_ge(sem["dz2"], 16)
                gp.tensor_tensor(sqb[:, blksl[2]], z[:, blksl[2]], z[:, blksl[2]],
                                 AL.mult).then_inc(sem["sg"])                             # 1
                gp.wait_ge(sem["dz3"], 16)
                gp.tensor_tensor(sqb[:, blksl[3]], z[:, blksl[3]], z[:, blksl[3]],
                                 AL.mult).then_inc(sem["sg"])                             # 2
                gp.wait_ge(sem["dout"], 16)

            @block.scalar
            def _(act):
                act.wait_ge(sem["dz0"], 16)
                act.square(sqb[:, blksl[0]], z[:, blksl[0]]).then_inc(sem["sa"])          # 1
                act.wait_ge(sem["dz1"], 16)
                act.square(sqb[:, blksl[1]], z[:, blksl[1]]).then_inc(sem["sa"])          # 2
                for i in range(4):                                                        # 3-6
                    act.wait_ge(sem["spe"], 2 * i + 2)
                    act.copy(qrow[:, blksl[i]], ps_q_t[i][:]).then_inc(sem["sa"])
                act.wait_ge(sem["sv"], 12)
                act.activation(std16[:], var16[:], AF.Sqrt).then_inc(sem["sa"])           # 7
                act.wait_ge(sem["sv"], 28)
                act.activation(u[:], z2n[:], AF.Square, bias=negone[:],
                               scale=a_sb[:]).then_inc(sem["sa"])                         # 8
                act.wait_ge(sem["spe"], 13)
                act.copy(q16sb[:], q16[:]).then_inc(sem["sa"])                            # 9

            @block.vector
            def _(dve):
                dve.memset(negone[:], -1.0).then_inc(sem["sv"])                           # 1
                dve.memset(ones_b[:], 1.0).then_inc(sem["sv"])                            # 2
                dve.wait_ge(sem["dz0"], 16)
                dve.tensor_copy(zb[:, blksl[0]], z[:, blksl[0]]).then_inc(sem["sv"])      # 3
                dve.wait_ge(sem["dz1"], 16)
                dve.tensor_copy(zb[:, blksl[1]], z[:, blksl[1]]).then_inc(sem["sv"])      # 4
                dve.wait_ge(sem["spe"], 1)
                dve.tensor_copy(sumrow[:, blksl[0]], ps_s_t[0][:]).then_inc(sem["sv"])    # 5
                dve.wait_ge(sem["spe"], 3)
                dve.tensor_copy(sumrow[:, blksl[1]], ps_s_t[1][:]).then_inc(sem["sv"])    # 6
                dve.wait_ge(sem["dz2"], 16)
                dve.tensor_copy(zb[:, blksl[2]], z[:, blksl[2]]).then_inc(sem["sv"])      # 7
                dve.wait_ge(sem["dz3"], 16)
                dve.tensor_copy(zb[:, blksl[3]], z[:, blksl[3]]).then_inc(sem["sv"])      # 8
                dve.wait_ge(sem["spe"], 5)
                dve.tensor_copy(sumrow[:, blksl[2]], ps_s_t[2][:]).then_inc(sem["sv"])    # 9
                dve.wait_ge(sem["spe"], 7)
                dve.tensor_copy(sumrow[:, blksl[3]], ps_s_t[3][:]).then_inc(sem["sv"])    # 10
                dve.wait_ge(sem["sscat"], 16)
                dve.scalar_tensor_tensor(
                    t1[:], sum16[:], 1.0 / (B * (B - 1.0)), sum16[:],
                    op0=AL.mult, op1=AL.mult).then_inc(sem["sv"])                         # 11
                dve.wait_ge(sem["qscat"], 16)
                dve.scalar_tensor_tensor(
                    var16[:], q16s[:], 1.0 / (B - 1.0), t1[:],
                    op0=AL.mult, op1=AL.subtract).then_inc(sem["sv"])                     # 12
                dve.wait_ge(sem["sa"], 7)
                dve.reciprocal(A_z[:], std16[:]).then_inc(sem["sv"])                      # 13
                # selector casts double as spacing before A_z is re-read
                dve.wait_ge(sem["sca"], 16)
                dve.tensor_copy(selz1b[:], selz1).then_inc(sem["sv"])                     # 14
                dve.tensor_copy(selz2b[:], selz2).then_inc(sem["sv"])                     # 15
                dve.tensor_copy(A_zb[:], A_z[:]).then_inc(sem["sv"])                      # 16
                dve.scalar_tensor_tensor(
                    C_zb[:], sum16[:], 1.0 / B, A_z[:],
                    op0=AL.mult, op1=AL.mult).then_inc(sem["sv"])                         # 17
                # normalize z1 after its bank's two matmuls; z2 after the rest
                dve.wait_ge(sem["spe"], 10)
                dve.tensor_tensor(tn1[:], z1r, psA1, AL.mult).then_inc(sem["sv"])         # 18
                dve.tensor_tensor(z1n[:], tn1[:], psC1, AL.subtract).then_inc(sem["sv"])  # 19
                dve.wait_ge(sem["spe"], 12)
                dve.tensor_tensor(tn2[:], z2r, psA2, AL.mult).then_inc(sem["sv"])         # 20
                dve.tensor_tensor(z2n[:], tn2[:], psC2, AL.subtract).then_inc(sem["sv"])  # 21
                dve.scalar_tensor_tensor(
                    w[:], z1n[:], 1.0, z2n[:], op0=AL.bypass, op1=AL.mult,
                    accum_out=colsD[:, 0:1]).then_inc(sem["sv"])                          # 22 R
                dve.scalar_tensor_tensor(
                    junkP[:], w[:], 1.0, w[:], op0=AL.bypass, op1=AL.mult,
                    accum_out=colsD[:, 1:2]).then_inc(sem["sv"])                          # 23 P
                dve.scalar_tensor_tensor(
                    junkA[:], z1n[:], 1.0, amask, op0=AL.bypass, op1=AL.mult,
                    accum_out=acol[:]).then_inc(sem["sv"])                                # 24
                dve.scalar_tensor_tensor(
                    junkV[:], z2n[:], 1.0, amask, op0=AL.bypass, op1=AL.mult,
                    accum_out=vcol[:]).then_inc(sem["sv"])                                # 25
                dve.scalar_tensor_tensor(
                    junkA2[:], z1n[:], 1.0, z1n[:], op0=AL.bypass, op1=AL.mult,
                    accum_out=colsD[:, 6:7]).then_inc(sem["sv"])                          # 26 Sa
                dve.scalar_tensor_tensor(
                    junkV2[:], z2n[:], 1.0, z2n[:], op0=AL.bypass, op1=AL.mult,
                    accum_out=colsD[:, 7:8]).then_inc(sem["sv"])                          # 27 Sv
                dve.stream_shuffle(a_sb[:], acol[:],
                                   [8 * (i // 8) for i in range(32)]).then_inc(sem["sv"])  # 28
                dve.wait_ge(sem["sa"], 8)
                dve.scalar_tensor_tensor(
                    junkQ[:], u[:], 1.0, u[:], op0=AL.bypass, op1=AL.mult,
                    accum_out=colsD[:, 2:3]).then_inc(sem["sv"])                          # 29 Q
                dve.scalar_tensor_tensor(
                    junkG[:], u[:], 1.0, amask, op0=AL.bypass, op1=AL.mult,
                    accum_out=colsD[:, 3:4]).then_inc(sem["sv"])                          # 30 gd
                dve.tensor_tensor(colsD[:, 4:5], acol[:], vcol[:],
                                  AL.mult).then_inc(sem["sv"])                            # 31 d
                dve.tensor_tensor(colsD[:, 5:6], acol[:], acol[:],
                                  AL.mult).then_inc(sem["sv"])                            # 32 a2
                # ---- finals ----
                R_ = q16sb[:, 0:1]
                P_ = q16sb[:, 1:2]
                Q_ = q16sb[:, 2:3]
                gd_ = q16sb[:, 3:4]
                d_ = q16sb[:, 4:5]
                a2_ = q16sb[:, 5:6]
                Sa_ = q16sb[:, 6:7]
                Sv_ = q16sb[:, 7:8]
                d2 = fin[:, 0:1]
                g4 = fin[:, 1:2]
                h = fin[:, 2:3]
                Tp = fin[:, 3:4]
                on1 = fin[:, 4:5]
                on2 = fin[:, 5:6]
                e1 = fin[:, 6:7]
                f1 = fin[:, 7:8]
                f2 = fin[:, 8:9]
                u1 = fin[:, 9:10]
                u2 = fin[:, 10:11]
                off = fin[:, 11:12]
                hm = fin[:, 12:13]
                Tp2 = fin[:, 13:14]
                dve.wait_ge(sem["sa"], 9)
                dve.tensor_tensor(d2, d_, d_, AL.mult).then_inc(sem["sv"])       # 33
                dve.tensor_tensor(g4, gd_, gd_, AL.mult).then_inc(sem["sv"])     # 34
                dve.tensor_scalar_add(hm, gd_, -1.0).then_inc(sem["sv"])         # 35
                dve.scalar_tensor_tensor(
                    Tp, R_, -2.0, P_, op0=AL.mult, op1=AL.add).then_inc(sem["sv"])  # 36
                dve.tensor_tensor(e1, Sa_, a2_, AL.subtract).then_inc(sem["sv"])    # 37
                dve.tensor_tensor(h, hm, hm, AL.mult).then_inc(sem["sv"])        # 38
                dve.tensor_tensor(f2, d2, Q_, AL.add).then_inc(sem["sv"])        # 39
                dve.tensor_scalar_add(Tp2, Tp, float(D)).then_inc(sem["sv"])     # 40
                dve.tensor_tensor(f1, e1, Sv_, AL.mult).then_inc(sem["sv"])      # 41
                dve.tensor_tensor(u2, f2, P_, AL.subtract).then_inc(sem["sv"])   # 42
                dve.scalar_tensor_tensor(
                    on1, gd_, -1.0, Tp2,
                    op0=AL.mult, op1=AL.add).then_inc(sem["sv"])                 # 43
                dve.tensor_tensor(u1, f1, g4, AL.subtract).then_inc(sem["sv"])   # 44
                dve.drain().then_inc(sem["sv"])                                  # 45
                dve.tensor_tensor(on2, on1, h, AL.add).then_inc(sem["sv"])       # 46
                dve.tensor_tensor(off, u1, u2, AL.add).then_inc(sem["sv"])       # 47
                dve.drain().then_inc(sem["sv"])                                  # 48
                dve.scalar_tensor_tensor(
                    loss16[:], off, LAM, on2,
                    op0=AL.mult, op1=AL.add).then_inc(sem["sv"])                 # 49
                dve.drain().then_inc(sem["sv"])                                  # 50
                dve.engine_nop().then_inc(sem["sv"])                             # 51

            @block.tensor
            def _(pe):
                pe.wait_ge(sem["sv"], 3)
                pe.matmul(ps_s_t[0][:], ones_b[:], zb[:, blksl[0]],
                          start=True, stop=True).then_inc(sem["spe"])                     # 1
                pe.wait_ge(sem["sa"], 1)
                pe.matmul(ps_q_t[0][:], ones_b[:], sqb[:, blksl[0]],
                          start=True, stop=True).then_inc(sem["spe"])                     # 2
                pe.wait_ge(sem["sv"], 4)
                pe.matmul(ps_s_t[1][:], ones_b[:], zb[:, blksl[1]],
                          start=True, stop=True).then_inc(sem["spe"])                     # 3
                pe.wait_ge(sem["sa"], 2)
                pe.matmul(ps_q_t[1][:], ones_b[:], sqb[:, blksl[1]],
                          start=True, stop=True).then_inc(sem["spe"])                     # 4
                pe.wait_ge(sem["sv"], 7)   # zb2; WAR s0-copy at sv5
                pe.matmul(ps_s_t[2][:], ones_b[:], zb[:, blksl[2]], start=True,
                          stop=True, skip_group_check=True).then_inc(sem["spe"])          # 5
                pe.wait_ge(sem["sg"], 1)   # sq2
                pe.wait_ge(sem["sa"], 3)   # WAR q0-copy
                pe.matmul(ps_q_t[2][:], ones_b[:], sqb[:, blksl[2]], start=True,
                          stop=True, skip_group_check=True).then_inc(sem["spe"])          # 6
                pe.wait_ge(sem["sv"], 8)   # zb3; WAR s1-copy at sv6
                pe.matmul(ps_s_t[3][:], ones_b[:], zb[:, blksl[3]], start=True,
                          stop=True, skip_group_check=True).then_inc(sem["spe"])          # 7
                pe.wait_ge(sem["sg"], 2)   # sq3
                pe.wait_ge(sem["sa"], 4)   # WAR q1-copy
                pe.matmul(ps_q_t[3][:], ones_b[:], sqb[:, blksl[3]], start=True,
                          stop=True, skip_group_check=True).then_inc(sem["spe"])          # 8
                # broadcasts (bf16): bank A then bank B
                pe.wait_ge(sem["sv"], 16)
                pe.matmul(psA1, selz1b[:], A_zb[:], start=True,
                          stop=True).then_inc(sem["spe"])                                 # 9
                pe.wait_ge(sem["sv"], 17)
                pe.matmul(psC1, selz1b[:], C_zb[:], start=True, stop=True,
                          skip_group_check=True).then_inc(sem["spe"])                     # 10
                pe.matmul(psA2, selz2b[:], A_zb[:], start=True, stop=True,
                          skip_group_check=True).then_inc(sem["spe"])                     # 11
                pe.matmul(psC2, selz2b[:], C_zb[:], start=True, stop=True,
                          skip_group_check=True).then_inc(sem["spe"])                     # 12
                # group reduce
                pe.wait_ge(sem["sv"], 32)
                pe.matmul(q16[:], gsel, colsD[:], start=True,
                          stop=True).then_inc(sem["spe"])                                 # 13

    return nc


def _host_inputs(z1, z2):
    """Per-core input maps (sharding glue)."""
    z1 = np.ascontiguousarray(z1, np.float32)
    z2 = np.ascontiguousarray(z2, np.float32)

    base = np.zeros((128, C_TOTAL), np.float32)
    for m in range(128):
        base[8 * (m // 8), C_EXPAND + m] = 1.0   # expand
        base[m, C_GSEL + m // 8] = 1.0           # gsel
        base[m % 8, C_SELZ1 + m] = 1.0           # selz1
        base[8 + m % 8, C_SELZ2 + m] = 1.0       # selz2

    in_maps = []
    for c in range(NCORES):
        rows = slice(c * SPC, (c + 1) * SPC)
        consts = base.copy()
        consts[:, C_Z1R:C_Z1R + 128] = \
            z1[rows].reshape(SPC, KCH, 128).reshape(128, 128)
        consts[:, C_Z2R:C_Z2R + 128] = \
            z2[rows].reshape(SPC, KCH, 128).reshape(128, 128)
        for s in range(SPC):
            consts[s * 8, C_AMASK + c * SPC + s] = 1.0
        in_maps.append({
            "z1": z1, "z2": z2,
            "consts": np.ascontiguousarray(consts),
        })
    return in_maps


_cached_nc = None


def run(z1, z2, trace=False, **kwargs):
    global _cached_nc
    if _cached_nc is None:
        _cached_nc = build_program()
    in_maps = _host_inputs(z1, z2)
    res = run_bass_kernel_spmd(
        _cached_nc, in_maps, core_ids=list(range(NCORES)), trace=trace, **kwargs)
    out = np.concatenate([res.results[c]["loss"][:, 0] for c in range(NCORES)])
    return out.astype(np.float32), res


def kernel(z1, z2):
    out, _ = run(z1, z2, trace=False)
    return out



# revision 15
# speedup vs baseline: 1.1179x; 1.1179x over previous
"""BarlowTwins-style loss kernel for Trainium2 (raw Bass), 8-core SPMD.

Math: the reference materializes a (B, D, D) per-sample cross-correlation
tensor, but the loss algebraically reduces to O(B*D) work.  With
z1n/z2n the per-dim (batch-)normalized inputs, and per sample b
(y = z2n[b,:], a = z1n[b,b], v = z2n[b,b], d = a*v, g2 = (d-1)^2):
    R  = sum(z1n*z2n);  P = sum((z1n*z2n)^2)
    Sa = sum(z1n^2);    Mk = sum(y^k)  (M2..M4 power sums)
    Q  = sum((a*y - 1)^4) = a^4*M4 - 4a^3*M3 + 6a^2*M2 - 4a*M1 + D
    on   = P - 2R + D - g2 + (g2-1)^2
    off  = (Sa - a^2)*M2 - P + d^2 + Q - g2^2
    loss = on + 0.005*off

Layout: data-parallel over batch; each core normalizes its own 16
samples in a rearranged [128 part = (sample, col-chunk), 128 free]
layout.  Per-dim batch stats (colsum/colsumsq of the full z1/z2) are
computed on the PE with shifted-window one-hot stationaries so chunk k's
stats land at partitions {32q+k}; a single 32-lane stream_shuffle then
broadcasts the fp32 normalization constants to all 128 partitions
(no PSUM->SBUF row copies, no scatter DMAs, no bf16 loss on A/C).

Raw Bass (explicit semaphores): the walrus build in this container only
supports a single sync-wait per instruction, which the Tile framework's
exit sequence violates; standalone wait_ge instructions compose freely.
"""

import sys
from contextlib import ExitStack

import numpy as np

for _p in ("/opt/trn_rl_repo",):
    if _p not in sys.path:
        sys.path.append(_p)

import concourse.bass as bass
import concourse.mybir as mybir
from concourse.bass_utils import run_bass_kernel_spmd

B, D = 128, 1024
NCORES = 8
SPC = B // NCORES  # 16 samples per core
LAM = 0.005

FP = mybir.dt.float32
BF = mybir.dt.bfloat16
AF = mybir.ActivationFunctionType
AL = mybir.AluOpType

# consts column layout
C_AMASK = 0    # 128 cols fp32 one-hot diag selector
C_GSEL = 128   # 16 cols group-reduce selector
C_WOFF = 144   # 10 cols off-diag weight row (rows 0:16)
C_WON = 154    # 5 cols on-diag weight row
C_TOTAL = 160

NPRE = 10      # PE prewarm dummy matmuls (HAM clock ramp)

S1 = 1.0 / float(np.sqrt(B * (B - 1.0)))  # t = (SUM*S1)^2 = SUM^2/(B(B-1))

# semaphore value maps (inc order within each engine)
SV = dict(c1a=1, c2a=2, c1b=3, c2b=4, var=5, C=6, shuf=7, tn=8, zn=9,
          R=10, P=11, M1=12, a=13, v=14, M3=15, d=16, a3=17, X=18,
          e1=19, f1=20, on=21, off=22, loss=23)
SA = dict(pre1=1, pre2=2, pre3=3, sq1a=4, sq1b=5, t=6, ln=7, A=8, Sa=9,
          M2=10, M4=11, cpA=12, cpV=13, cpRPS=14, cpP=15, cpM2=16,
          a2=17, a4=18, g2=19, h=20, d2=21, g4=22)
SG = dict(win=5, negone=6, ones=7, eps=8, actail=9, sq2a=10, sq2b=11)


def build_program():
    nc = bass.Bass("TRN2", debug=False, num_devices=NCORES,
                   detect_race_conditions=False)

    z1_d = nc.dram_tensor("z1", [B, D], FP, kind="ExternalInput")
    z2_d = nc.dram_tensor("z2", [B, D], FP, kind="ExternalInput")
    zr_d = nc.dram_tensor("zr", [128, 256], FP, kind="ExternalInput")
    consts_d = nc.dram_tensor("consts", [128, C_TOTAL], FP, kind="ExternalInput")
    loss_d = nc.dram_tensor("loss", [SPC, 1], FP, kind="ExternalOutput")

    ctx = ExitStack()
    with ctx:
        sem = {n: ctx.enter_context(nc.semaphore(n)) for n in
               ["s1a", "s1b", "s2a", "s2b", "szr", "scst", "sout",
                "sv", "sa", "sg", "spe"]}

        def sb(name, shape, dtype=FP):
            return ctx.enter_context(nc.sbuf_tensor(name, shape, dtype))

        z = sb("z", [128, 2048])            # fp32 staging: z1 0:1024, z2 1024:2048
        zb = sb("zb", [128, 4096], BF)      # z1|z2|sq1|sq2, 1024 cols each
        zr = sb("zr_s", [128, 256])         # rearranged own-slice z1|z2
        ca = sb("ca", [128, C_TOTAL])
        tT = sb("tT", [128, 256])
        varT = sb("varT", [128, 256])
        AC = sb("AC", [128, 512])           # A 0:256 | C 256:512 (parts 0:104)
        ACbc = sb("ACbc", [128, 512])       # broadcast to all partitions
        tn = sb("tn", [128, 256])
        zn = sb("zn", [128, 256])           # z1n 0:128 | z2n 128:256
        w = sb("w", [128, 128])
        y2 = sb("y2", [128, 128])
        junkD = sb("junkD", [128, 128])
        junkG = sb("junkG", [128, 128])
        junkA = sb("junkA", [128, 128])
        colsD = sb("colsD", [128, 12])      # R P Sa M4 M3 M2 M1 v a
        negone = sb("negone", [128, 1])
        epsb = sb("epsb", [128, 1])
        winbuf = sb("winbuf", [128, 112], BF)
        dumact = sb("dumact", [128, 1])
        fin = sb("fin", [16, 24])
        junkF = sb("junkF", [16, 16])
        loss16 = sb("loss16", [16, 1])

        stats = ctx.enter_context(nc.psum_tensor("stats", [128, 512], FP))
        G = ctx.enter_context(nc.psum_tensor("G", [16, 16], FP))
        dum = ctx.enter_context(nc.psum_tensor("dum", [128, 512], FP))

        one_ap = nc.const_aps.aps[(FP, 1.0)]

        # fin cols: 0 a4 | 1 a3 | 2 a2 | 3 a | 4 v | 5:9 t4 t3 t2 t1 |
        #           9 d | 10 f1 | 11 P | 12 d2 | 13 g4 | 14 ones | 15 g2 |
        #           16 h | 17 R | 18 P2 | 19 Sa | 20 offc | 21 onc | 22 M2
        # (scalar G columns come via ACT copies: DVE free-1 PSUM reads at
        #  non-32B offsets return zero on HW)
        amask = ca[:, C_AMASK:C_AMASK + 128]
        gsel = ca[:, C_GSEL:C_GSEL + 16]
        woff = ca[0:16, C_WOFF:C_WOFF + 10]
        won = ca[0:16, C_WON:C_WON + 5]

        with nc.Block() as block:

            @block.sync
            def _(sync):
                sync.dma_start(z[:, 0:512], z1_d[:, 0:512]).then_inc(sem["s1a"], 16)
                sync.dma_start(z[:, 512:1024], z1_d[:, 512:1024]).then_inc(sem["s1b"], 16)
                sync.dma_start(zr[:], zr_d[:]).then_inc(sem["szr"], 16)
                sync.dma_start(ca[:], consts_d[:]).then_inc(sem["scst"], 16)
                sync.wait_ge(sem["sv"], SV["loss"])
                sync.dma_start(loss_d[:], loss16[:]).then_inc(sem["sout"], 16)

            @block.scalar
            def _(act):
                act.dma_start(z[:, 1024:1536], z2_d[:, 0:512]).then_inc(sem["s2a"], 16)
                act.dma_start(z[:, 1536:2048], z2_d[:, 512:1024]).then_inc(sem["s2b"], 16)
                # activation-table preloads during the DMA shadow
                act.activation(dumact[:], one_ap, AF.Square).then_inc(sem["sa"])   # 1
                act.activation(dumact[:], one_ap, AF.Ln).then_inc(sem["sa"])       # 2
                act.activation(dumact[:], one_ap, AF.Exp).then_inc(sem["sa"])      # 3
                act.wait_ge(sem["s1a"], 16)
                act.square(zb[:, 2048:2560], z[:, 0:512]).then_inc(sem["sa"])      # 4
                act.wait_ge(sem["s1b"], 16)
                act.square(zb[:, 2560:3072], z[:, 512:1024]).then_inc(sem["sa"])   # 5
                act.wait_ge(sem["spe"], 8)
                act.activation(tT[0:104, :], stats[0:104, 0:256], AF.Square,
                               scale=S1).then_inc(sem["sa"])                       # 6
                act.wait_ge(sem["sv"], SV["var"])
                # +eps keeps the unused (non {32q+k}) rows finite: Ln(0+eps)
                act.wait_ge(sem["sg"], SG["actail"])
                act.activation(tT[0:104, :], varT[0:104, :],
                               AF.Ln, bias=epsb[0:104, :]).then_inc(sem["sa"])     # 7
                act.activation(AC[0:104, 0:256], tT[0:104, :], AF.Exp,
                               scale=-0.5).then_inc(sem["sa"])                     # 8
                act.wait_ge(sem["sv"], SV["zn"])
                act.activation(junkA[:], zn[:, 0:128], AF.Square,
                               accum_out=colsD[:, 2:3]).then_inc(sem["sa"])        # 9 Sa
                act.activation(y2[:], zn[:, 128:256], AF.Square,
                               accum_out=colsD[:, 5:6]).then_inc(sem["sa"])        # 10 M2
                act.activation(junkA[:], y2[:], AF.Square,
                               accum_out=colsD[:, 3:4]).then_inc(sem["sa"])        # 11 M4
                # ---- finals (scalar side) ----
                act.wait_ge(sem["spe"], 9)
                act.copy(fin[:, 3:4], G[0:16, 8:9]).then_inc(sem["sa"])            # 12 a
                act.copy(fin[:, 4:5], G[0:16, 7:8]).then_inc(sem["sa"])            # 13 v
                act.copy(fin[:, 17:20], G[0:16, 0:3]).then_inc(sem["sa"])          # 14 R,P2,Sa
                act.copy(fin[:, 11:12], G[0:16, 1:2]).then_inc(sem["sa"])          # 15 P
                act.copy(fin[:, 22:23], G[0:16, 5:6]).then_inc(sem["sa"])          # 16 M2
                act.activation(fin[:, 2:3], G[0:16, 8:9],
                               AF.Square).then_inc(sem["sa"])                      # 17 a2
                act.activation(fin[:, 0:1], fin[:, 2:3],
                               AF.Square).then_inc(sem["sa"])                      # 18 a4
                act.wait_ge(sem["sv"], SV["d"])
                act.wait_ge(sem["sg"], SG["negone"])
                act.activation(fin[:, 15:16], fin[:, 9:10], AF.Square,
                               bias=negone[0:16, :]).then_inc(sem["sa"])           # 19 g2
                act.activation(fin[:, 16:17], fin[:, 15:16], AF.Square,
                               bias=negone[0:16, :]).then_inc(sem["sa"])           # 20 h
                act.activation(fin[:, 12:13], fin[:, 9:10],
                               AF.Square).then_inc(sem["sa"])                      # 21 d2
                act.activation(fin[:, 13:14], fin[:, 15:16],
                               AF.Square).then_inc(sem["sa"])                      # 22 g4

            @block.vector
            def _(dve):
                dve.wait_ge(sem["s1a"], 16)
                dve.tensor_copy(zb[:, 0:512], z[:, 0:512]).then_inc(sem["sv"])       # 1
                dve.wait_ge(sem["s2a"], 16)
                dve.tensor_copy(zb[:, 1024:1536], z[:, 1024:1536]).then_inc(sem["sv"])  # 2
                dve.wait_ge(sem["s1b"], 16)
                dve.tensor_copy(zb[:, 512:1024], z[:, 512:1024]).then_inc(sem["sv"])  # 3
                dve.wait_ge(sem["s2b"], 16)
                dve.tensor_copy(zb[:, 1536:2048], z[:, 1536:2048]).then_inc(sem["sv"])  # 4
                dve.wait_ge(sem["sa"], SA["t"])
                dve.scalar_tensor_tensor(
                    varT[0:104, :], stats[0:104, 256:512], 1.0 / (B - 1.0),
                    tT[0:104, :], op0=AL.mult, op1=AL.subtract).then_inc(sem["sv"])  # 5
                dve.wait_ge(sem["sa"], SA["A"])
                dve.scalar_tensor_tensor(
                    AC[0:104, 256:512], stats[0:104, 0:256], 1.0 / B,
                    AC[0:104, 0:256], op0=AL.mult, op1=AL.mult).then_inc(sem["sv"])  # 6 C
                dve.wait_ge(sem["sg"], SG["actail"])
                dve.stream_shuffle(ACbc[:], AC[:],
                                   [i % 8 for i in range(32)]).then_inc(sem["sv"])   # 7
                dve.wait_ge(sem["szr"], 16)
                dve.tensor_tensor(tn[:], zr[:], ACbc[:, 0:256],
                                  AL.mult).then_inc(sem["sv"])                       # 8
                dve.tensor_tensor(zn[:], tn[:], ACbc[:, 256:512],
                                  AL.subtract).then_inc(sem["sv"])                   # 9
                dve.scalar_tensor_tensor(
                    w[:], zn[:, 0:128], 1.0, zn[:, 128:256], op0=AL.bypass,
                    op1=AL.mult, accum_out=colsD[:, 0:1]).then_inc(sem["sv"])        # 10 R
                dve.scalar_tensor_tensor(
                    junkD[:], w[:], 1.0, w[:], op0=AL.bypass,
                    op1=AL.mult, accum_out=colsD[:, 1:2]).then_inc(sem["sv"])        # 11 P
                dve.scalar_tensor_tensor(
                    junkD[:], zn[:, 128:256], 1.0, zn[:, 128:256], op0=AL.bypass,
                    op1=AL.max, accum_out=colsD[:, 6:7]).then_inc(sem["sv"])         # 12 M1
                dve.wait_ge(sem["scst"], 16)
                dve.scalar_tensor_tensor(
                    junkD[:], zn[:, 0:128], 1.0, amask, op0=AL.bypass,
                    op1=AL.mult, accum_out=colsD[:, 8:9]).then_inc(sem["sv"])        # 13 a
                dve.scalar_tensor_tensor(
                    junkD[:], zn[:, 128:256], 1.0, amask, op0=AL.bypass,
                    op1=AL.mult, accum_out=colsD[:, 7:8]).then_inc(sem["sv"])        # 14 v
                dve.wait_ge(sem["sa"], SA["M2"])
                dve.scalar_tensor_tensor(
                    junkD[:], y2[:], 1.0, zn[:, 128:256], op0=AL.bypass,
                    op1=AL.mult, accum_out=colsD[:, 4:5]).then_inc(sem["sv"])        # 15 M3
                # ---- finals (vector side) ----
                dve.wait_ge(sem["sa"], SA["cpV"])
                dve.tensor_tensor(fin[:, 9:10], fin[:, 3:4], fin[:, 4:5],
                                  AL.mult).then_inc(sem["sv"])                       # 16 d
                dve.wait_ge(sem["sa"], SA["a2"])
                dve.tensor_tensor(fin[:, 1:2], fin[:, 2:3], fin[:, 3:4],
                                  AL.mult).then_inc(sem["sv"])                       # 17 a3
                dve.drain()
                dve.wait_ge(sem["sa"], SA["a4"])
                dve.tensor_tensor(fin[:, 5:9], fin[:, 0:4], G[0:16, 3:7],
                                  AL.mult).then_inc(sem["sv"])                       # 18 t4..t1
                dve.tensor_tensor(fin[:, 23:24], fin[:, 19:20], fin[:, 2:3],
                                  AL.subtract).then_inc(sem["sv"])                   # 19 e1
                dve.drain()
                dve.tensor_tensor(fin[:, 10:11], fin[:, 23:24], fin[:, 22:23],
                                  AL.mult).then_inc(sem["sv"])                       # 20 f1
                dve.wait_ge(sem["sa"], SA["g4"])
                dve.wait_ge(sem["sg"], SG["ones"])
                dve.scalar_tensor_tensor(
                    junkF[:, 10:15], fin[:, 14:19], 1.0, won, op0=AL.bypass,
                    op1=AL.mult, accum_out=fin[:, 21:22]).then_inc(sem["sv"])        # 21 on
                dve.drain()
                dve.scalar_tensor_tensor(
                    junkF[:, 0:10], fin[:, 5:15], 1.0, woff, op0=AL.bypass,
                    op1=AL.mult, accum_out=fin[:, 20:21]).then_inc(sem["sv"])        # 22 off
                dve.drain()
                dve.scalar_tensor_tensor(
                    loss16[:], fin[:, 20:21], LAM, fin[:, 21:22],
                    op0=AL.mult, op1=AL.add).then_inc(sem["sv"])                     # 22

            @block.gpsimd
            def _(gp):
                gp.memset(winbuf[:], 0.0).then_inc(sem["sg"])                        # 1
                gp.memset(winbuf[:, 7:8], 1.0).then_inc(sem["sg"])                   # 2
                gp.memset(winbuf[:, 39:40], 1.0).then_inc(sem["sg"])                 # 3
                gp.memset(winbuf[:, 71:72], 1.0).then_inc(sem["sg"])                 # 4
                gp.memset(winbuf[:, 103:104], 1.0).then_inc(sem["sg"])               # 5
                gp.memset(negone[:], -1.0).then_inc(sem["sg"])                       # 6
                gp.memset(fin[:, 14:15], 1.0).then_inc(sem["sg"])                    # 7
                gp.memset(epsb[:], 1e-20).then_inc(sem["sg"])                        # 8
                gp.memset(AC[96:128, :], 0.0).then_inc(sem["sg"])                    # 9
                gp.wait_ge(sem["s2a"], 16)
                gp.tensor_tensor(zb[:, 3072:3584], z[:, 1024:1536], z[:, 1024:1536],
                                 AL.mult).then_inc(sem["sg"])                        # 10
                gp.wait_ge(sem["s2b"], 16)
                gp.tensor_tensor(zb[:, 3584:4096], z[:, 1536:2048], z[:, 1536:2048],
                                 AL.mult).then_inc(sem["sg"])                        # 11
                gp.wait_ge(sem["sout"], 16)

            @block.tensor
            def _(pe):
                pe.wait_ge(sem["sg"], 5)
                win_rep = bass.AP(winbuf, 0, [[112, 128], [0, 4], [1, 104]])
                for _ in range(NPRE):
                    pe.matmul(dum[0:104, 0:416], winbuf[:, 0:104], win_rep,
                              start=True, stop=True, skip_group_check=True)
                for k in range(8):
                    if k == 0:
                        pe.wait_ge(sem["sv"], SV["c2a"])
                        pe.wait_ge(sem["sa"], SA["sq1a"])
                        pe.wait_ge(sem["sg"], SG["sq2a"])
                    elif k == 4:
                        pe.wait_ge(sem["sv"], SV["c2b"])
                        pe.wait_ge(sem["sa"], SA["sq1b"])
                        pe.wait_ge(sem["sg"], SG["sq2b"])
                    rhs_k = bass.AP(zb, k * 128, [[4096, 128], [1024, 4], [1, 128]])
                    pe.matmul(stats[0:104, :], winbuf[:, 7 - k:111 - k], rhs_k,
                              start=(k == 0), stop=(k == 7),
                              skip_group_check=True).then_inc(sem["spe"])        # 1-8
                pe.wait_ge(sem["sv"], SV["M3"])
                pe.wait_ge(sem["sa"], SA["M4"])
                pe.wait_ge(sem["scst"], 16)
                pe.matmul(G[0:16, 0:9], gsel, colsD[:, 0:9], start=True,
                          stop=True).then_inc(sem["spe"])                        # 9

    return nc


def _host_inputs(z1, z2):
    """Per-core input maps (sharding glue)."""
    z1 = np.ascontiguousarray(z1, np.float32)
    z2 = np.ascontiguousarray(z2, np.float32)

    base = np.zeros((128, C_TOTAL), np.float32)
    for p in range(128):
        base[p, C_GSEL + p // 8] = 1.0
    # weight rows (same for all 16 loss rows)
    woff = [1.0, -4.0, 6.0, -4.0, 0.0, 1.0, -1.0, 1.0, -1.0, float(D)]
    won = [float(D), -1.0, 1.0, -2.0, 1.0]
    base[0:16, C_WOFF:C_WOFF + 10] = np.array(woff, np.float32)
    base[0:16, C_WON:C_WON + 5] = np.array(won, np.float32)

    in_maps = []
    for c in range(NCORES):
        rows = slice(c * SPC, (c + 1) * SPC)
        consts = base.copy()
        for s in range(SPC):
            consts[s * 8, C_AMASK + c * SPC + s] = 1.0
        zr = np.concatenate([z1[rows].reshape(128, 128),
                             z2[rows].reshape(128, 128)], axis=1)
        in_maps.append({
            "z1": z1, "z2": z2,
            "zr": np.ascontiguousarray(zr),
            "consts": np.ascontiguousarray(consts),
        })
    return in_maps


_cached_nc = None


def run(z1, z2, trace=False, **kwargs):
    global _cached_nc
    if _cached_nc is None:
        _cached_nc = build_program()
    in_maps = _host_inputs(z1, z2)
    res = run_bass_kernel_spmd(
        _cached_nc, in_maps, core_ids=list(range(NCORES)), trace=trace, **kwargs)
    out = np.concatenate([res.results[c]["loss"][:, 0] for c in range(NCORES)])
    return out.astype(np.float32), res


def kernel(z1, z2):
    out, _ = run(z1, z2, trace=False)
    return out
